# revision 1
# baseline (speedup 1.0000x reference)
"""EmmaAttention EMA-merge kernel for 8 Trainium2 NeuronCores.

Computation (per node n, head h):
    beta  = clip(1 - inv_w * agg_n[n], 0, 1)
    max_m = max(max_a, his_m)
    p     = exp(his_m - max_m) * beta
    q     = exp(max_a - max_m)
    t     = max(p + q, 1.0)
    out[n,h,:] = his_x[n,h,:] * (p/t) + x[n,h,:] * (q/t)

Pure elementwise over N -> shard N across the 8 cores, no communication.

Per-core layout: Nc = 25000 nodes on P = 125 partitions, 200 nodes per
partition (node = partition*200 + g).  Per-(node,head) scalars p/t, q/t are
precomputed once into SBUF ([125, 1600]), then the [125, G*512] main-loop
tiles multiply against them via stride-0 broadcast APs over D=64.
"""

import numpy as np

N, H, D = 200000, 8, 64
HD = H * D
NCORES = 8
NC_SHARD = N // NCORES  # 25000 nodes per core
P = 125                 # SBUF partitions used (25000 = 125 * 200)
NPP = NC_SHARD // P     # 200 nodes per partition
G = 10                  # nodes-per-partition per main-loop tile
NT = NPP // G           # 20 main-loop tiles
FD = G * HD             # 5120 f32 free-dim elements per tile
SH = G * H              # 80 (node,head) scalars per tile per partition

_CACHE = {}


def _build_program():
    from concourse import mybir, tile, bacc
    from concourse.bass import ts

    nc = bacc.Bacc(trn_type="TRN2")
    f32 = mybir.dt.float32

    x = nc.dram_tensor("x", (NC_SHARD, H, D), f32, kind="ExternalInput")
    max_a = nc.dram_tensor("max_a", (NC_SHARD, H), f32, kind="ExternalInput")
    his_x = nc.dram_tensor("his_x", (NC_SHARD, H, D), f32, kind="ExternalInput")
    his_m = nc.dram_tensor("his_m", (NC_SHARD, H), f32, kind="ExternalInput")
    agg_n = nc.dram_tensor("agg_n", (NC_SHARD,), f32, kind="ExternalInput")
    inv_w = nc.dram_tensor("inv_w", (1,), f32, kind="ExternalInput")
    out = nc.dram_tensor("out", (NC_SHARD, H, D), f32, kind="ExternalOutput")

    x3 = x[:].rearrange("(p g) h d -> p g (h d)", p=P)     # [125, 200, 512]
    hx3 = his_x[:].rearrange("(p g) h d -> p g (h d)", p=P)
    o3 = out[:].rearrange("(p g) h d -> p g (h d)", p=P)
    ma2 = max_a[:].rearrange("(p g) h -> p (g h)", p=P)    # [125, 1600]
    hm2 = his_m[:].rearrange("(p g) h -> p (g h)", p=P)
    an2 = agg_n[:].rearrange("(p g) -> p g", p=P)          # [125, 200]

    Alu = mybir.AluOpType
    Act = mybir.ActivationFunctionType

    with tile.TileContext(nc) as tc:
        with (
            tc.tile_pool(name="small", bufs=1) as sp,
            tc.tile_pool(name="big", bufs=3) as bp,
        ):
            ma_t = sp.tile((P, NPP * H), f32)
            nc.sync.dma_start(ma_t[:], ma2)
            hm_t = sp.tile((P, NPP * H), f32)
            nc.sync.dma_start(hm_t[:], hm2)
            an_t = sp.tile((P, NPP), f32)
            nc.sync.dma_start(an_t[:], an2)
            iw_t = sp.tile((P, 1), f32)
            nc.sync.dma_start(iw_t[:], inv_w[:].to_broadcast((P, 1)))

            mm_t = sp.tile((P, NPP * H), f32)
            p_t = sp.tile((P, NPP * H), f32)
            q_t = sp.tile((P, NPP * H), f32)
            bt_t = sp.tile((P, NPP), f32)
            niw_t = sp.tile((P, 1), f32)

            # p/t and q/t scalars, [125, 1600] (g-major, h-minor)
            nc.vector.tensor_max(mm_t[:], ma_t[:], hm_t[:])
            nc.vector.tensor_sub(hm_t[:], hm_t[:], mm_t[:])
            nc.vector.tensor_sub(ma_t[:], ma_t[:], mm_t[:])
            nc.scalar.activation(p_t[:], hm_t[:], Act.Exp)
            nc.scalar.activation(q_t[:], ma_t[:], Act.Exp)
            # beta = clip(1 - inv_w*agg_n, 0, 1), then p *= beta (bcast over h)
            nc.scalar.mul(niw_t[:], iw_t[:], -1.0)
            nc.vector.tensor_scalar(bt_t[:], an_t[:], niw_t[:], 1.0, Alu.mult, Alu.add)
            nc.vector.tensor_scalar(bt_t[:], bt_t[:], 0.0, 1.0, Alu.max, Alu.min)
            p3 = p_t[:].rearrange("p (g h) -> p g h", h=H)
            nc.vector.tensor_mul(p3, p3, bt_t[:, :, None].to_broadcast((P, NPP, H)))
            # r = 1 / max(p + q, 1)
            nc.vector.tensor_add(mm_t[:], p_t[:], q_t[:])
            nc.vector.tensor_scalar_max(mm_t[:], mm_t[:], 1.0)
            nc.vector.reciprocal(mm_t[:], mm_t[:])
            nc.vector.tensor_mul(p_t[:], p_t[:], mm_t[:])
            nc.vector.tensor_mul(q_t[:], q_t[:], mm_t[:])

            # main loop: out = his_x * p + x * q, p/q broadcast over D
            for t in range(NT):
                x_t = bp.tile((P, FD), f32)
                nc.sync.dma_start(x_t[:], x3[:, ts(t, G), :])
                h_t = bp.tile((P, FD), f32)
                nc.sync.dma_start(h_t[:], hx3[:, ts(t, G), :])

                h3 = h_t[:].rearrange("p (s d) -> p s d", d=D)
                xx3 = x_t[:].rearrange("p (s d) -> p s d", d=D)
                pb = p_t[:, ts(t, SH)][:, :, None].to_broadcast((P, SH, D))
                qb = q_t[:, ts(t, SH)][:, :, None].to_broadcast((P, SH, D))
                nc.vector.tensor_mul(h3, h3, pb)
                nc.vector.tensor_mul(xx3, xx3, qb)
                nc.vector.tensor_add(h_t[:], h_t[:], x_t[:])
                nc.sync.dma_start(o3[:, ts(t, G), :], h_t[:])

    nc.finalize()
    return nc


def _get_program():
    if "nc" not in _CACHE:
        _CACHE["nc"] = _build_program()
    return _CACHE["nc"]


def _make_in_maps(x, max_a, his_x, his_m, agg_n, inv_w):
    x = np.ascontiguousarray(x, dtype=np.float32)
    max_a = np.ascontiguousarray(max_a, dtype=np.float32)
    his_x = np.ascontiguousarray(his_x, dtype=np.float32)
    his_m = np.ascontiguousarray(his_m, dtype=np.float32)
    agg_n = np.ascontiguousarray(agg_n, dtype=np.float32)
    inv_w = np.ascontiguousarray(inv_w, dtype=np.float32)
    in_maps = []
    for c in range(NCORES):
        s = slice(c * NC_SHARD, (c + 1) * NC_SHARD)
        in_maps.append(
            {
                "x": x[s],
                "max_a": max_a[s],
                "his_x": his_x[s],
                "his_m": his_m[s],
                "agg_n": agg_n[s],
                "inv_w": inv_w,
            }
        )
    return in_maps


def kernel_run(x, max_a, his_x, his_m, agg_n, inv_w, **run_kwargs):
    """Run on HW; returns (full_output, BassKernelResults)."""
    from concourse.bass_utils import run_bass_kernel_spmd

    nc = _get_program()
    in_maps = _make_in_maps(x, max_a, his_x, his_m, agg_n, inv_w)
    res = run_bass_kernel_spmd(nc, in_maps, core_ids=list(range(NCORES)), **run_kwargs)
    full = np.concatenate([res.results[c]["out"] for c in range(NCORES)], axis=0)
    return full, res


def kernel(x, max_a, his_x, his_m, agg_n, inv_w):
    full, _ = kernel_run(x, max_a, his_x, his_m, agg_n, inv_w)
    return full


# revision 2
# speedup vs baseline: 1.1571x; 1.1571x over previous
"""EmmaAttention EMA-merge kernel for 8 Trainium2 NeuronCores.

Computation (per node n, head h):
    beta  = clip(1 - inv_w * agg_n[n], 0, 1)
    max_m = max(max_a, his_m)
    p     = exp(his_m - max_m) * beta
    q     = exp(max_a - max_m)
    t     = max(p + q, 1.0)
    out[n,h,:] = his_x[n,h,:] * (p/t) + x[n,h,:] * (q/t)

Pure elementwise over N -> shard N across the 8 cores, no communication.

Per-core layout: Nc = 25000 nodes on P = 125 partitions, 200 nodes per
partition (node = partition*200 + g).  Per-(node,head) scalars p/t, q/t are
precomputed once into SBUF ([125, 1600]), then the [125, G*512] main-loop
tiles multiply against them via stride-0 broadcast APs over D=64.
"""

import numpy as np

N, H, D = 200000, 8, 64
HD = H * D
NCORES = 8
NC_SHARD = N // NCORES  # 25000 nodes per core
P = 125                 # SBUF partitions used (25000 = 125 * 200)
NPP = NC_SHARD // P     # 200 nodes per partition
G = 10                  # nodes-per-partition per main-loop tile
NT = NPP // G           # 20 main-loop tiles
FD = G * HD             # 5120 f32 free-dim elements per tile
SH = G * H              # 80 (node,head) scalars per tile per partition

_CACHE = {}


def _build_program():
    from concourse import mybir, tile, bacc
    from concourse.bass import ts

    nc = bacc.Bacc(trn_type="TRN2")
    f32 = mybir.dt.float32

    x = nc.dram_tensor("x", (NC_SHARD, H, D), f32, kind="ExternalInput")
    max_a = nc.dram_tensor("max_a", (NC_SHARD, H), f32, kind="ExternalInput")
    his_x = nc.dram_tensor("his_x", (NC_SHARD, H, D), f32, kind="ExternalInput")
    his_m = nc.dram_tensor("his_m", (NC_SHARD, H), f32, kind="ExternalInput")
    agg_n = nc.dram_tensor("agg_n", (NC_SHARD,), f32, kind="ExternalInput")
    inv_w = nc.dram_tensor("inv_w", (1,), f32, kind="ExternalInput")
    out = nc.dram_tensor("out", (NC_SHARD, H, D), f32, kind="ExternalOutput")

    x3 = x[:].rearrange("(p g) h d -> p g (h d)", p=P)     # [125, 200, 512]
    hx3 = his_x[:].rearrange("(p g) h d -> p g (h d)", p=P)
    o3 = out[:].rearrange("(p g) h d -> p g (h d)", p=P)
    ma2 = max_a[:].rearrange("(p g) h -> p (g h)", p=P)    # [125, 1600]
    hm2 = his_m[:].rearrange("(p g) h -> p (g h)", p=P)
    an2 = agg_n[:].rearrange("(p g) -> p g", p=P)          # [125, 200]

    Alu = mybir.AluOpType
    Act = mybir.ActivationFunctionType

    with tile.TileContext(nc) as tc:
        with (
            tc.tile_pool(name="small", bufs=1) as sp,
            tc.tile_pool(name="big", bufs=3) as bp,
        ):
            ma_t = sp.tile((P, NPP * H), f32)
            nc.sync.dma_start(ma_t[:], ma2)
            hm_t = sp.tile((P, NPP * H), f32)
            nc.sync.dma_start(hm_t[:], hm2)
            an_t = sp.tile((P, NPP), f32)
            nc.sync.dma_start(an_t[:], an2)
            iw_t = sp.tile((P, 1), f32)
            nc.sync.dma_start(iw_t[:], inv_w[:].to_broadcast((P, 1)))

            mm_t = sp.tile((P, NPP * H), f32)
            p_t = sp.tile((P, NPP * H), f32)
            q_t = sp.tile((P, NPP * H), f32)
            bt_t = sp.tile((P, NPP), f32)
            niw_t = sp.tile((P, 1), f32)

            # p/t and q/t scalars, [125, 1600] (g-major, h-minor)
            nc.vector.tensor_max(mm_t[:], ma_t[:], hm_t[:])
            nc.vector.tensor_sub(hm_t[:], hm_t[:], mm_t[:])
            nc.vector.tensor_sub(ma_t[:], ma_t[:], mm_t[:])
            nc.scalar.activation(p_t[:], hm_t[:], Act.Exp)
            nc.scalar.activation(q_t[:], ma_t[:], Act.Exp)
            # beta = clip(1 - inv_w*agg_n, 0, 1), then p *= beta (bcast over h)
            nc.scalar.mul(niw_t[:], iw_t[:], -1.0)
            nc.vector.tensor_scalar(bt_t[:], an_t[:], niw_t[:], 1.0, Alu.mult, Alu.add)
            nc.vector.tensor_scalar(bt_t[:], bt_t[:], 0.0, 1.0, Alu.max, Alu.min)
            p3 = p_t[:].rearrange("p (g h) -> p g h", h=H)
            nc.vector.tensor_mul(p3, p3, bt_t[:, :, None].to_broadcast((P, NPP, H)))
            # r = 1 / max(p + q, 1)
            nc.vector.tensor_add(mm_t[:], p_t[:], q_t[:])
            nc.vector.tensor_scalar_max(mm_t[:], mm_t[:], 1.0)
            nc.vector.reciprocal(mm_t[:], mm_t[:])
            nc.vector.tensor_mul(p_t[:], p_t[:], mm_t[:])
            nc.vector.tensor_mul(q_t[:], q_t[:], mm_t[:])

            # main loop: out = his_x * p + x * q, p/q broadcast over D.
            # Spread DMA traffic across the three independent DMA queue
            # rows (sync HWDGE / scalar HWDGE / gpsimd SWDGE) so more SDMA
            # engines run in parallel (each row only gets ~5 of the 16
            # engines; one row alone tops out at ~135 GB/s).
            for t in range(NT):
                x_t = bp.tile((P, FD), f32)
                nc.sync.dma_start(x_t[:], x3[:, ts(t, G), :])
                h_t = bp.tile((P, FD), f32)
                nc.scalar.dma_start(h_t[:], hx3[:, ts(t, G), :])

                h3 = h_t[:].rearrange("p (s d) -> p s d", d=D)
                xx3 = x_t[:].rearrange("p (s d) -> p s d", d=D)
                pb = p_t[:, ts(t, SH)][:, :, None].to_broadcast((P, SH, D))
                qb = q_t[:, ts(t, SH)][:, :, None].to_broadcast((P, SH, D))
                nc.vector.tensor_mul(h3, h3, pb)
                nc.vector.tensor_mul(xx3, xx3, qb)
                nc.vector.tensor_add(h_t[:], h_t[:], x_t[:])
                nc.gpsimd.dma_start(o3[:, ts(t, G), :], h_t[:])

    nc.finalize()
    return nc


def _get_program():
    if "nc" not in _CACHE:
        _CACHE["nc"] = _build_program()
    return _CACHE["nc"]


def _make_in_maps(x, max_a, his_x, his_m, agg_n, inv_w):
    x = np.ascontiguousarray(x, dtype=np.float32)
    max_a = np.ascontiguousarray(max_a, dtype=np.float32)
    his_x = np.ascontiguousarray(his_x, dtype=np.float32)
    his_m = np.ascontiguousarray(his_m, dtype=np.float32)
    agg_n = np.ascontiguousarray(agg_n, dtype=np.float32)
    inv_w = np.ascontiguousarray(inv_w, dtype=np.float32)
    in_maps = []
    for c in range(NCORES):
        s = slice(c * NC_SHARD, (c + 1) * NC_SHARD)
        in_maps.append(
            {
                "x": x[s],
                "max_a": max_a[s],
                "his_x": his_x[s],
                "his_m": his_m[s],
                "agg_n": agg_n[s],
                "inv_w": inv_w,
            }
        )
    return in_maps


def kernel_run(x, max_a, his_x, his_m, agg_n, inv_w, **run_kwargs):
    """Run on HW; returns (full_output, BassKernelResults)."""
    from concourse.bass_utils import run_bass_kernel_spmd

    nc = _get_program()
    in_maps = _make_in_maps(x, max_a, his_x, his_m, agg_n, inv_w)
    res = run_bass_kernel_spmd(nc, in_maps, core_ids=list(range(NCORES)), **run_kwargs)
    full = np.concatenate([res.results[c]["out"] for c in range(NCORES)], axis=0)
    return full, res


def kernel(x, max_a, his_x, his_m, agg_n, inv_w):
    full, _ = kernel_run(x, max_a, his_x, his_m, agg_n, inv_w)
    return full


# revision 3
# speedup vs baseline: 1.4647x; 1.2659x over previous
"""EmmaAttention EMA-merge kernel for 8 Trainium2 NeuronCores.

Computation (per node n, head h):
    beta  = clip(1 - inv_w * agg_n[n], 0, 1)
    max_m = max(max_a, his_m)
    p     = exp(his_m - max_m) * beta
    q     = exp(max_a - max_m)
    t     = max(p + q, 1.0)
    out[n,h,:] = his_x[n,h,:] * (p/t) + x[n,h,:] * (q/t)

Pure elementwise over N -> shard N across the 8 cores, no communication.

Per-core layout: Nc = 25000 nodes on P = 125 partitions, 200 nodes per
partition (node = partition*200 + g).  Per-(node,head) scalars p/t, q/t are
precomputed once into SBUF ([125, 1600]), then the [125, G*512] main-loop
tiles multiply against them via stride-0 broadcast APs over D=64.
"""

import numpy as np

N, H, D = 200000, 8, 64
HD = H * D
NCORES = 8
NC_SHARD = N // NCORES  # 25000 nodes per core
P = 125                 # SBUF partitions used (25000 = 125 * 200)
NPP = NC_SHARD // P     # 200 nodes per partition
G = 10                  # nodes-per-partition per main-loop tile
NT = NPP // G           # 20 main-loop tiles
FD = G * HD             # 5120 f32 free-dim elements per tile
SH = G * H              # 80 (node,head) scalars per tile per partition

_CACHE = {}


def _build_program():
    from concourse import mybir, tile, bacc
    from concourse.bass import ts

    nc = bacc.Bacc(trn_type="TRN2")
    f32 = mybir.dt.float32

    x = nc.dram_tensor("x", (NC_SHARD, H, D), f32, kind="ExternalInput")
    max_a = nc.dram_tensor("max_a", (NC_SHARD, H), f32, kind="ExternalInput")
    his_x = nc.dram_tensor("his_x", (NC_SHARD, H, D), f32, kind="ExternalInput")
    his_m = nc.dram_tensor("his_m", (NC_SHARD, H), f32, kind="ExternalInput")
    agg_n = nc.dram_tensor("agg_n", (NC_SHARD,), f32, kind="ExternalInput")
    inv_w = nc.dram_tensor("inv_w", (1,), f32, kind="ExternalInput")
    out = nc.dram_tensor("out", (NC_SHARD, H, D), f32, kind="ExternalOutput")

    x3 = x[:].rearrange("(p g) h d -> p g (h d)", p=P)     # [125, 200, 512]
    hx3 = his_x[:].rearrange("(p g) h d -> p g (h d)", p=P)
    o3 = out[:].rearrange("(p g) h d -> p g (h d)", p=P)
    ma2 = max_a[:].rearrange("(p g) h -> p (g h)", p=P)    # [125, 1600]
    hm2 = his_m[:].rearrange("(p g) h -> p (g h)", p=P)
    an2 = agg_n[:].rearrange("(p g) -> p g", p=P)          # [125, 200]

    Alu = mybir.AluOpType
    Act = mybir.ActivationFunctionType

    with tile.TileContext(nc) as tc:
        with (
            tc.tile_pool(name="small", bufs=1) as sp,
            tc.tile_pool(name="big", bufs=3) as bp,
        ):
            ma_t = sp.tile((P, NPP * H), f32)
            nc.sync.dma_start(ma_t[:], ma2)
            hm_t = sp.tile((P, NPP * H), f32)
            nc.sync.dma_start(hm_t[:], hm2)
            an_t = sp.tile((P, NPP), f32)
            nc.sync.dma_start(an_t[:], an2)
            iw_t = sp.tile((P, 1), f32)
            nc.sync.dma_start(iw_t[:], inv_w[:].to_broadcast((P, 1)))

            mm_t = sp.tile((P, NPP * H), f32)
            p_t = sp.tile((P, NPP * H), f32)
            q_t = sp.tile((P, NPP * H), f32)
            bt_t = sp.tile((P, NPP), f32)
            niw_t = sp.tile((P, 1), f32)

            # p/t and q/t scalars, [125, 1600] (g-major, h-minor)
            nc.vector.tensor_max(mm_t[:], ma_t[:], hm_t[:])
            nc.vector.tensor_sub(hm_t[:], hm_t[:], mm_t[:])
            nc.vector.tensor_sub(ma_t[:], ma_t[:], mm_t[:])
            nc.scalar.activation(p_t[:], hm_t[:], Act.Exp)
            nc.scalar.activation(q_t[:], ma_t[:], Act.Exp)
            # beta = clip(1 - inv_w*agg_n, 0, 1), then p *= beta (bcast over h)
            nc.scalar.mul(niw_t[:], iw_t[:], -1.0)
            nc.vector.tensor_scalar(bt_t[:], an_t[:], niw_t[:], 1.0, Alu.mult, Alu.add)
            nc.vector.tensor_scalar(bt_t[:], bt_t[:], 0.0, 1.0, Alu.max, Alu.min)
            p3 = p_t[:].rearrange("p (g h) -> p g h", h=H)
            nc.vector.tensor_mul(p3, p3, bt_t[:, :, None].to_broadcast((P, NPP, H)))
            # r = 1 / max(p + q, 1)
            nc.vector.tensor_add(mm_t[:], p_t[:], q_t[:])
            nc.vector.tensor_scalar_max(mm_t[:], mm_t[:], 1.0)
            nc.vector.reciprocal(mm_t[:], mm_t[:])
            nc.vector.tensor_mul(p_t[:], p_t[:], mm_t[:])
            nc.vector.tensor_mul(q_t[:], q_t[:], mm_t[:])

            # main loop: out = his_x * p + x * q, p/q broadcast over D.
            # Spread DMA traffic across the three independent DMA queue
            # rows (sync HWDGE / scalar HWDGE / gpsimd SWDGE) so more SDMA
            # engines run in parallel (each row only gets ~5 of the 16
            # engines; one row alone tops out at ~135 GB/s).
            for t in range(NT):
                x_t = bp.tile((P, FD), f32)
                nc.gpsimd.dma_start(x_t[:], x3[:, ts(t, G), :])
                h_t = bp.tile((P, FD), f32)
                nc.gpsimd.dma_start(h_t[:], hx3[:, ts(t, G), :])

                h3 = h_t[:].rearrange("p (s d) -> p s d", d=D)
                xx3 = x_t[:].rearrange("p (s d) -> p s d", d=D)
                pb = p_t[:, ts(t, SH)][:, :, None].to_broadcast((P, SH, D))
                qb = q_t[:, ts(t, SH)][:, :, None].to_broadcast((P, SH, D))
                nc.vector.tensor_mul(h3, h3, pb)
                nc.vector.tensor_mul(xx3, xx3, qb)
                nc.vector.tensor_add(h_t[:], h_t[:], x_t[:])
                nc.gpsimd.dma_start(o3[:, ts(t, G), :], h_t[:])

    nc.finalize()
    return nc


def _get_program():
    if "nc" not in _CACHE:
        _CACHE["nc"] = _build_program()
    return _CACHE["nc"]


def _make_in_maps(x, max_a, his_x, his_m, agg_n, inv_w):
    x = np.ascontiguousarray(x, dtype=np.float32)
    max_a = np.ascontiguousarray(max_a, dtype=np.float32)
    his_x = np.ascontiguousarray(his_x, dtype=np.float32)
    his_m = np.ascontiguousarray(his_m, dtype=np.float32)
    agg_n = np.ascontiguousarray(agg_n, dtype=np.float32)
    inv_w = np.ascontiguousarray(inv_w, dtype=np.float32)
    in_maps = []
    for c in range(NCORES):
        s = slice(c * NC_SHARD, (c + 1) * NC_SHARD)
        in_maps.append(
            {
                "x": x[s],
                "max_a": max_a[s],
                "his_x": his_x[s],
                "his_m": his_m[s],
                "agg_n": agg_n[s],
                "inv_w": inv_w,
            }
        )
    return in_maps


def kernel_run(x, max_a, his_x, his_m, agg_n, inv_w, **run_kwargs):
    """Run on HW; returns (full_output, BassKernelResults)."""
    from concourse.bass_utils import run_bass_kernel_spmd

    nc = _get_program()
    in_maps = _make_in_maps(x, max_a, his_x, his_m, agg_n, inv_w)
    res = run_bass_kernel_spmd(nc, in_maps, core_ids=list(range(NCORES)), **run_kwargs)
    full = np.concatenate([res.results[c]["out"] for c in range(NCORES)], axis=0)
    return full, res


def kernel(x, max_a, his_x, his_m, agg_n, inv_w):
    full, _ = kernel_run(x, max_a, his_x, his_m, agg_n, inv_w)
    return full
